# revision 38
# baseline (speedup 1.0000x reference)
"""Causal attention (B=4,H=16,S=2048,D=64) on 8 NeuronCores via Bass/Tile.

Per core = 8 heads; per head, 2 q-blocks of 1024 processed in 512-col
chunks against a [65,1024] PSUM numerator+denominator accumulator.

- Scores: 64-contraction fp16 matmuls issued as row-tiled concurrent
  pairs — even k-tiles' K^T lives in SBUF partitions 0:64
  (tile_position (0,0)), odd k-tiles' in 64:128 ((64,0)); Q^T is
  duplicated into both partition halves.  Disjoint row groups let the
  PE run both matmuls of a pair simultaneously (~2x score throughput
  vs full-height 128-contraction with a zero-padded half).  Each pair
  writes one [128,1024] PSUM tile (even tile cols 0:512, odd 512:1024).
- Causal stair: folded into the fast-exp — scalar_tensor_tensor
  computes st*A + stair with a per-slot stair bias ([2,640] layout:
  slot 1 carries a -1000 preamble over the even tile's extra columns,
  so the pair's exp is ONE DVE instruction and stale-PSUM gap columns
  saturate to fp8 +0.0).  Only the two exact ACT tiles per head keep
  the rank-128 -60000 mask matmul.
- exp (with a 2^-4 rescale so values fit fp8e4m3): exact on Scalar
  for the first two k-tiles of q-block 0 chunk 0; everywhere else a
  one-instruction-per-pair Schraudolph fast-exp writing fp8 bit
  patterns via saturating uint8 convert — diag pairs on DVE (stair
  bias), off-diag pairs statically load-balanced between ACT and DVE.
- PV: fp8e4m3 DoubleRow over k-tile pairs (contraction 256, V tiles
  padded to an 80-col pitch); the two exact tiles use fp16 PV.  PVs
  trail the score pipeline by PIPE pairs so exp latency is hidden.
- Copy-out split ACT/DVE by the load balancer; host divides by the
  denominator row and transposes back.
- Warm-up matmuls stream a zero fodder region so every score-PSUM
  bank holds bounded values before the first stair fast-exp reads a
  gap column.
"""
import os
import sys

sys.path.insert(0, "/opt/trn_rl_repo")

import numpy as np

B, H, S, D = 4, 16, 2048, 64
NCORES = 8
HPC = (B * H) // NCORES        # heads per core = 8
QB = 1024                      # q-block width (acc/copy granularity)
CH = 512                       # score/exp/PV chunk width (1 PSUM bank)
NQB = S // QB                  # q blocks per head = 2
V8K = 80                       # fp8 V k-tile pitch (DoubleRow LDW
                               # needs tile stride % 16 == 0)
NKT = S // 128                 # 128-wide k-tiles per head = 16
V8COLS = NKT * V8K             # 16*80 = 1280 (fp8 V, all k-tiles)
V16COLS = 2 * (D + 1)          # 130 (fp16 V, k-tiles 0-1 only)
KTC = 1024                     # kt cols per head (even top / odd bottom)
QTC = 2048                     # qt cols per head (Q^T duplicated)
# CMB pair layout: ktA | ktB | qtA | qtB | v16A | v16B
OFF_KTA, OFF_KTB = 0, KTC
OFF_QTA, OFF_QTB = 2 * KTC, 2 * KTC + QTC
OFF_V16A = 2 * KTC + 2 * QTC
OFF_V16B = OFF_V16A + V16COLS
PAIR_COLS = OFF_V16B + V16COLS  # 6404
NPAIR = HPC // 2               # 4
SCALE = 1.0 / 8.0              # 1/sqrt(D)
PVB = 3                        # PV burst size; PVs trail scores by
                               # >= PVB pairs so exp latency is hidden

# All E values carry a 2^-4 factor (cancels in numerator/denominator):
# raw scores reach ~55, and exp(55/8)=963 would overflow fp8e4m3's 448
# max (the uint8 fast-exp would wrap into NaN/negative patterns).
EXP_BIAS = float(-4.0 * np.log(2.0))  # exp(x*SCALE + EXP_BIAS)
# Schraudolph fast-exp constants (fp8e4m3 pattern via saturating uint8,
# ~7.3% max rel err; stair-masked scores saturate to 0 = fp8 +0.0).
FE8_A = float(8.0 * np.log2(np.e) * SCALE)
FE8_B = float(56.0 - 0.37 - 32.0)
MASK_B = -1000.0               # stair bias above the diagonal
A16 = True                     # exact ACT exp for k-tiles 0,1 of qb0

# engine cost model (ns): t = cols * NS_COL + NS_FIX
ACT_NS_COL, ACT_NS_FIX = 0.84, 290.0
DVE_NS_COL, DVE_NS_FIX = 1.05, 160.0

last_exec_time_ns = None

_prog_cache = {}


def _install_trace_hook():
    """Inject antenv.axon_hooks (missing from this image) so trace=True can
    capture NTFF profiles. Degrades silently if anything is unavailable."""
    import types

    try:
        import antenv

        if "antenv.axon_hooks" in sys.modules:
            return
        mod = types.ModuleType("antenv.axon_hooks")
        state = {"hook": None}
        mod.set_axon_ntff_profile_hook = lambda h: state.__setitem__("hook", h)
        mod.get_axon_ntff_profile_hook = lambda: state["hook"]
        sys.modules["antenv.axon_hooks"] = mod
        antenv.axon_hooks = mod
        from trn_agent_boot.trn_boot import _ntff_profile_via_ctypes

        hook = _ntff_profile_via_ctypes("/opt/axon/libaxon_pjrt.so")
        if hook is not None:
            mod.set_axon_ntff_profile_hook(hook)
    except Exception:
        pass


def _chunk_pairs(qb, c):
    """Tile pairs for chunk c of q-block qb: (ke, sge, sgo, diag)."""
    qc0 = qb * QB + c * CH
    kmax = qc0 // 128 + 4
    out = []
    for ke in range(0, kmax, 2):
        sge = max(0, 128 * ke - qc0)
        sgo = max(0, 128 * (ke + 1) - qc0)
        out.append((ke, sge, sgo, 128 * ke >= qc0))
    return out


def _plan_exp_engines():
    """Static engine plan.

    plan[(head, qb, c, ke)] -> 'a16' | 'act' | 'dve' per tile PAIR;
    cplan[(head, qb)] -> cols assigned to ACT for the copy-out (rest
    DVE).  Diag pairs are pinned to DVE (the stair bias lives in the
    fused scalar_tensor_tensor); off-diag pairs and the copy split
    balance modeled finish times.
    """
    load = {"act": 0.0, "dve": 0.0}

    def cost(eng, cols):
        return (cols * ACT_NS_COL + ACT_NS_FIX if eng == "act"
                else cols * DVE_NS_COL + DVE_NS_FIX)

    plan = {}
    cplan = {}
    for head in range(HPC):
        for qb in range(NQB):
            for c in range(QB // CH):
                for ke, sge, sgo, diag in _chunk_pairs(qb, c):
                    if A16 and qb == 0 and c == 0 and ke == 0:
                        plan[(head, qb, c, ke)] = "a16"
                        load["act"] += (cost("act", CH)
                                        + cost("act", CH - 128))
                    elif diag:
                        plan[(head, qb, c, ke)] = "dve"
                        load["dve"] += cost("dve", 2 * (CH - sge))
                    elif load["act"] + cost("act", 2 * CH) <= \
                            load["dve"] + cost("dve", 2 * CH):
                        plan[(head, qb, c, ke)] = "act"
                        load["act"] += cost("act", 2 * CH)
                    else:
                        plan[(head, qb, c, ke)] = "dve"
                        load["dve"] += cost("dve", 2 * CH)
            # copy-out split: give ACT enough cols to equalize
            gap = load["dve"] - load["act"]
            aa = int((QB * DVE_NS_COL + gap + DVE_NS_FIX - ACT_NS_FIX)
                     / (ACT_NS_COL + DVE_NS_COL))
            aa = max(0, min(QB, (aa // 64) * 64))
            cplan[(head, qb)] = aa
            if aa > 0:
                load["act"] += cost("act", aa)
            if aa < QB:
                load["dve"] += cost("dve", QB - aa)
    return plan, cplan


def _build_program():
    import concourse.bass as bass  # noqa: F401
    import concourse.mybir as mybir
    import concourse.tile as tile
    from concourse import bacc

    F16 = mybir.dt.float16
    F32 = mybir.dt.float32
    F8 = mybir.dt.float8e4
    U8 = mybir.dt.uint8
    EXP = mybir.ActivationFunctionType.Exp
    MULT = mybir.AluOpType.mult
    ADD = mybir.AluOpType.add
    DR = mybir.MatmulPerfMode.DoubleRow

    plan, cplan = _plan_exp_engines()

    nc = bacc.Bacc()
    CMB = nc.declare_dram_parameter(
        "CMB", [128, NPAIR * PAIR_COLS], F16, isOutput=False
    )
    VA8 = nc.declare_dram_parameter(
        "VA8", [128, HPC * V8COLS], F8, isOutput=False
    )
    TRI = nc.declare_dram_parameter("TRI", [128, 1408], F16, isOutput=False)
    OUT = nc.declare_dram_parameter("OUT", [HPC, D + 1, S], F16, isOutput=True)

    with tile.TileContext(nc) as tc:
        with (
            tc.tile_pool(name="cmbp", bufs=2) as cmbp,
            tc.tile_pool(name="singles", bufs=1) as singles,
            tc.tile_pool(name="etp16", bufs=2) as etp16,
            tc.tile_pool(name="etp8", bufs=8) as etp8,
            tc.tile_pool(name="obp", bufs=2) as obp,
            tc.tile_pool(name="stp", bufs=3, space="PSUM") as stp,
            tc.tile_pool(name="accp", bufs=1, space="PSUM") as accp,
        ):
            biast = singles.tile([128, 1], F32, tag="ebias")
            nc.gpsimd.memset(biast, EXP_BIAS)
            # memset-built zero fodder: the PE warm-up needs no DMA
            zw = singles.tile([128, 640], F16, tag="zw")
            nc.gpsimd.memset(zw, 0.0)
            trib = singles.tile([128, 1408], F16, tag="tri")
            # 0/1 causal stair for the two exact ACT tiles (applied
            # as a GpSimd multiply on the fp16 E tile)
            stairm = trib[:, 0:128]
            # per-slot stair bias for the fused DVE fast-exp:
            # slot 0: stair then FE8_B; slot 1: -1000 preamble (covers
            # the even tile's extra cols), stair, then FE8_B.
            stair2 = trib[:, 128:1408].rearrange("p (s c) -> p s c", s=2)

            va8b = singles.tile([128, HPC * V8COLS], F8, tag="va8")

            # PE warm-up: the HAM clock gate only un-throttles (1.2 ->
            # 2.4 GHz) under sustained matmul activity; streamed zeros
            # also leave every score-PSUM bank bounded before the first
            # stair fast-exp reads a stale gap column.
            for wi in range(9):
                wt = stp.tile([128, 2 * CH], F32, tag="st",
                              name=f"warm{wi}")
                nc.tensor.matmul(wt[:, 0:CH], zw[:, 0:128],
                                 zw[:, 128:640], start=True, stop=True)
                nc.tensor.matmul(wt[:, CH:2 * CH], zw[:, 0:128],
                                 zw[:, 128:640], start=True, stop=True)

            # stage pair 0 so head 0's first chunk can start early:
            # both heads' kt, head A's qb0 Q, the fp16 V tiles and the
            # first heads' fp8 V, then the remainder.
            cmbs = [cmbp.tile([128, PAIR_COLS], F16, tag="cmb",
                              name=f"cmb{p}") for p in range(NPAIR)]
            nc.sync.dma_start(out=cmbs[0][:, 0:2 * KTC],
                              in_=CMB[:, 0:2 * KTC])
            nc.sync.dma_start(out=trib, in_=TRI[:])
            nc.gpsimd.dma_start(
                out=cmbs[0][:, OFF_QTA:OFF_QTA + QB],
                in_=CMB[:, OFF_QTA:OFF_QTA + QB])
            nc.gpsimd.dma_start(
                out=cmbs[0][:, OFF_V16A:PAIR_COLS],
                in_=CMB[:, OFF_V16A:PAIR_COLS])
            nc.scalar.dma_start(
                out=va8b[:, 0:2 * V8COLS], in_=VA8[:, 0:2 * V8COLS]
            )
            for c0, c1 in (
                (OFF_QTA + QB, OFF_QTB),       # qtA qb1 cols
                (OFF_QTB, OFF_QTB + QB),       # qtB qb0
                (OFF_QTB + QB, OFF_V16A),      # qtB qb1
            ):
                nc.sync.dma_start(out=cmbs[0][:, c0:c1],
                                  in_=CMB[:, c0:c1])
            nc.sync.dma_start(
                out=va8b[:, 2 * V8COLS:], in_=VA8[:, 2 * V8COLS:]
            )

            # copy-out closures deferred into the NEXT block's pair
            # stream so they never head-of-line-block the engines'
            # strict FIFO queues while waiting for the PV flush
            carry = []

            for pair in range(NPAIR):
                cmb = cmbs[pair]
                for sub in range(2):
                    if sub == 1 and pair + 1 < NPAIR:
                        # prefetch the next pair on the idle GpSimd
                        # DMA queue: issued at the second head so it
                        # never contends with this pair's own staging,
                        # overlaps ~16us of compute, and is not stuck
                        # behind OUT drains on the Sync queue
                        nc.gpsimd.dma_start(
                            out=cmbs[pair + 1],
                            in_=CMB[:, (pair + 1) * PAIR_COLS:
                                    (pair + 2) * PAIR_COLS],
                        )
                    head = 2 * pair + sub
                    kt = cmb[:, OFF_KTA + sub * KTC:OFF_KTA + (sub + 1) * KTC]
                    qt = cmb[:, OFF_QTA + sub * QTC:OFF_QTA + (sub + 1) * QTC]
                    v16off = OFF_V16A + sub * V16COLS
                    va16 = cmb[:, v16off:v16off + V16COLS].rearrange(
                        "p (t c) -> p t c", t=2
                    )
                    va8 = va8b[:, head * V8COLS:(head + 1) * V8COLS
                               ].rearrange("p (t c) -> p t c", t=NKT)

                    for qb in range(NQB):
                        q0 = QB * qb
                        acc = accp.tile(
                            [D + 1, QB], F32, tag="acc",
                            name=f"acc_h{head}_qb{qb}",
                        )
                        e16s = {}
                        pend = []      # pv work awaiting issue

                        def do_st_pair(c, ke, sge, sgo, diag,
                                       first, last):
                            """Row-tiled concurrent score pair + exp."""
                            qc0 = q0 + c * CH
                            tcol = 64 * ke
                            kt_e = kt[0:64, tcol:tcol + 128]
                            kt_o = kt[64:128, tcol:tcol + 128]
                            nm = f"h{head}_q{qb}{c}_k{ke}"
                            st = stp.tile([128, 2 * CH], F32, tag="st",
                                          name=f"st_{nm}")
                            nc.tensor.matmul(
                                st[:, sge:CH], kt_e,
                                qt[0:64, qc0 + sge:qc0 + CH],
                                start=True, stop=True,
                            )
                            nc.tensor.matmul(
                                st[:, CH + sgo:2 * CH], kt_o,
                                qt[64:128, qc0 + sgo:qc0 + CH],
                                start=True, stop=True,
                            )
                            st3 = st.rearrange("p (s c) -> p s c", s=2)
                            if A16 and qb == 0 and c == 0 and ke == 0:
                                # exact tiles: unmasked ACT exp, then
                                # the causal stair applied as a 0/1
                                # multiply on the idle GpSimd engine
                                # (keeps the rank-128 mask matmul off
                                # the PE)
                                for ki in (0, 1):
                                    sg = 128 * ki
                                    et = etp16.tile(
                                        [128, CH], F16, tag="et16",
                                        name=f"et_h{head}_k{ki}",
                                    )
                                    nc.scalar.activation(
                                        et[:, sg:CH],
                                        st[:, ki * CH + sg:(ki + 1) * CH],
                                        EXP, bias=biast, scale=SCALE,
                                    )
                                    nc.gpsimd.tensor_tensor(
                                        et[:, sg:sg + 128],
                                        et[:, sg:sg + 128],
                                        stairm, MULT,
                                    )
                                    e16s[ki] = (et, sg)
                                return
                            e8 = etp8.tile([128, 2, CH], F8, tag="et8",
                                           name=f"e8_{nm}")
                            eng = plan[(head, qb, c, ke)]
                            if diag:
                                # fused stair fast-exp over both slots:
                                # (st*A)+stair2, saturating uint8 ->
                                # fp8 pattern; slot 1's -1000 preamble
                                # zeroes the gap columns.
                                nc.vector.scalar_tensor_tensor(
                                    e8[:, :, sge:CH].bitcast(U8),
                                    st3[:, :, sge:CH],
                                    FE8_A, stair2[:, :, 0:CH - sge],
                                    MULT, ADD,
                                )
                            elif eng == "dve":
                                nc.vector.tensor_scalar(
                                    e8.bitcast(U8), st3,
                                    FE8_A, FE8_B, MULT, ADD,
                                )
                            else:
                                nc.scalar.activation(
                                    e8, st3, EXP,
                                    bias=biast, scale=SCALE,
                                )
                            pend.append((c, ke, sge, e8, first, last))

                        def do_pv16():
                            for ki in (0, 1):
                                et, sg = e16s.pop(ki)
                                nc.tensor.matmul(
                                    acc[:, sg:CH], va16[:, ki, :],
                                    et[:, sg:CH],
                                    start=(ki == 0), stop=False,
                                )

                        def do_pv8(item):
                            c, ke, sge, e8, first, last = item
                            c0 = c * CH
                            va_k = va8[:, ke:ke + 2, 0:D + 1]
                            nc.tensor.matmul(
                                acc[:, c0 + sge:c0 + CH], va_k,
                                e8[:, :, sge:CH],
                                start=first, stop=last,
                                perf_mode=DR,
                            )

                        pcount = 0
                        for c in range(QB // CH):
                            cps = _chunk_pairs(qb, c)
                            for t, (ke, sge, sgo, diag) in enumerate(cps):
                                # first PV of the chunk clears PSUM;
                                # in qb0 chunk0 that's the PV16 pair
                                first = t == 0 and not (
                                    A16 and qb == 0 and c == 0)
                                last = t == len(cps) - 1
                                do_st_pair(c, ke, sge, sgo, diag,
                                           first, last)
                                pcount += 1
                                # the deferred copy must be emitted
                                # before this block's first PV writes
                                # the (single-buffered) acc: before
                                # PV16 (pair 2) in qb0, before the
                                # first PV burst (pair 6) in qb1
                                if carry and pcount == 2:
                                    carry.pop(0)()
                                if A16 and qb == 0 and c == 0 and ke == 2:
                                    do_pv16()
                                if len(pend) >= 2 * PVB:
                                    for _ in range(PVB):
                                        do_pv8(pend.pop(0))
                        while pend:
                            do_pv8(pend.pop(0))

                        def emit_copy(acc=acc, head=head, qb=qb, q0=q0):
                            ob = obp.tile(
                                [D + 1, QB], F16, tag="ob",
                                name=f"ob_h{head}_qb{qb}",
                            )
                            aa = cplan[(head, qb)]
                            if aa > 0:
                                nc.scalar.copy(ob[:, 0:aa], acc[:, 0:aa])
                            if aa < QB:
                                nc.vector.tensor_copy(
                                    ob[:, aa:QB], acc[:, aa:QB]
                                )
                            nc.sync.dma_start(
                                out=OUT[head, :, q0:q0 + QB], in_=ob,
                            )
                        carry.append(emit_copy)
            while carry:
                carry.pop(0)()
    nc.finalize()
    return nc


def _get_program():
    if "nc" not in _prog_cache:
        _prog_cache["nc"] = _build_program()
    return _prog_cache["nc"]


def kernel(q, k, v, mask):
    global last_exec_time_ns
    q = np.asarray(q, dtype=np.float32)
    k = np.asarray(k, dtype=np.float32)
    v = np.asarray(v, dtype=np.float32)
    mask = np.asarray(mask).astype(bool)

    # This kernel specializes the causal (lower-triangular) mask from the
    # module; for any other mask fall back to a host reference.
    tril = np.tril(np.ones((S, S), dtype=bool))
    if mask.shape != (1, 1, S, S) or not np.array_equal(mask[0, 0], tril):
        scores = np.einsum("bhqd,bhkd->bhqk", q, k) / np.sqrt(np.float32(D))
        scores = np.where(mask, scores, -np.inf)
        m = scores.max(-1, keepdims=True)
        e = np.exp(scores - m)
        return (np.einsum("bhqk,bhkd->bhqd", e / e.sum(-1, keepdims=True), v)
                .astype(np.float32))

    _install_trace_hook()
    import ml_dtypes
    from concourse.bass_utils import run_bass_kernel_spmd

    nc = _get_program()

    F8NP = ml_dtypes.float8_e4m3fn
    qf = q.reshape(B * H, S, D).astype(np.float16)
    kf = k.reshape(B * H, S, D).astype(np.float16)
    vf = v.reshape(B * H, S, D).astype(np.float16)

    tri_np = np.zeros((128, 1408), dtype=np.float16)
    # 0/1 causal stair: keep k_rel (p) <= q_rel (j)
    pp = np.arange(128)[:, None]
    jj = np.arange(128)[None, :]
    tri_np[:, 0:128] = (pp <= jj).astype(np.float16)
    # stair bias slots: [p, j] = FE8_B if p <= j else MASK_B
    p = np.arange(128)[:, None]
    j = np.arange(128)[None, :]
    stair = np.where(p <= j, np.float16(FE8_B), np.float16(MASK_B))
    s0 = np.full((128, 640), np.float16(FE8_B), dtype=np.float16)
    s0[:, 0:128] = stair
    s1 = np.full((128, 640), np.float16(FE8_B), dtype=np.float16)
    s1[:, 0:128] = np.float16(MASK_B)
    s1[:, 128:256] = stair
    tri_np[:, 128:768] = s0
    tri_np[:, 768:1408] = s1

    def _kt_pack(h):
        kth = np.zeros((128, KTC), dtype=np.float16)
        kT = kf[h].T  # [64, 2048]
        for t in range(NKT // 2):
            kth[0:64, 128 * t:128 * (t + 1)] = kT[:, 256 * t:256 * t + 128]
            kth[64:128, 128 * t:128 * (t + 1)] = \
                kT[:, 256 * t + 128:256 * t + 256]
        return kth

    in_maps = []
    for core in range(NCORES):
        pairs = []
        va8s = []
        for p_ in range(NPAIR):
            hA = core * HPC + 2 * p_
            hB = hA + 1
            ktA, ktB = _kt_pack(hA), _kt_pack(hB)
            qtA = np.concatenate([qf[hA].T, qf[hA].T], axis=0)  # dup
            qtB = np.concatenate([qf[hB].T, qf[hB].T], axis=0)
            v16s = []
            for h in (hA, hB):
                vt = vf[h].reshape(NKT, 128, D).transpose(1, 0, 2)
                va = np.concatenate(
                    [vt, np.ones((128, NKT, 1), dtype=np.float16)], axis=2
                )  # [128, NKT, 65]
                v16s.append(va[:, 0:2, :].reshape(128, V16COLS))
                va8p = np.zeros((128, NKT, V8K), dtype=F8NP)
                va8p[:, :, 0:D + 1] = va.astype(F8NP)
                va8s.append(va8p.reshape(128, V8COLS))
            pairs.append(
                np.concatenate([ktA, ktB, qtA, qtB, v16s[0], v16s[1]],
                               axis=1)
            )
        cmb = np.ascontiguousarray(np.concatenate(pairs, axis=1))
        va8 = np.ascontiguousarray(np.concatenate(va8s, axis=1))
        in_maps.append({"CMB": cmb, "VA8": va8, "TRI": tri_np})

    trace = bool(os.environ.get("ATTN_TRACE"))
    res = run_bass_kernel_spmd(
        nc, in_maps, list(range(NCORES)), trace=trace
    )
    last_exec_time_ns = res.exec_time_ns

    out = np.empty((B * H, S, D), dtype=np.float32)
    for core in range(NCORES):
        acc = res.results[core]["OUT"].astype(np.float32)  # [HPC, 65, S]
        o = acc[:, :D, :] / acc[:, D:D + 1, :]
        out[core * HPC:(core + 1) * HPC] = o.transpose(0, 2, 1)
    return out.reshape(B, H, S, D)


# revision 39
# speedup vs baseline: 1.0089x; 1.0089x over previous
"""Causal attention (B=4,H=16,S=2048,D=64) on 8 NeuronCores via Bass/Tile.

Per core = 8 heads; per head, 2 q-blocks of 1024 processed in 512-col
chunks against a [65,1024] PSUM numerator+denominator accumulator.

- Scores: 64-contraction fp16 matmuls issued as row-tiled concurrent
  pairs — even k-tiles' K^T lives in SBUF partitions 0:64
  (tile_position (0,0)), odd k-tiles' in 64:128 ((64,0)); Q^T is
  duplicated into both partition halves.  Disjoint row groups let the
  PE run both matmuls of a pair simultaneously (~2x score throughput
  vs full-height 128-contraction with a zero-padded half).  Each pair
  writes one [128,1024] PSUM tile (even tile cols 0:512, odd 512:1024).
- Causal stair: folded into the fast-exp — scalar_tensor_tensor
  computes st*A + stair with a per-slot stair bias ([2,640] layout:
  slot 1 carries a -1000 preamble over the even tile's extra columns,
  so the pair's exp is ONE DVE instruction and stale-PSUM gap columns
  saturate to fp8 +0.0).  Only the two exact ACT tiles per head keep
  the rank-128 -60000 mask matmul.
- exp (with a 2^-4 rescale so values fit fp8e4m3): exact on Scalar
  for the first two k-tiles of q-block 0 chunk 0; everywhere else a
  one-instruction-per-pair Schraudolph fast-exp writing fp8 bit
  patterns via saturating uint8 convert — diag pairs on DVE (stair
  bias), off-diag pairs statically load-balanced between ACT and DVE.
- PV: fp8e4m3 DoubleRow over k-tile pairs (contraction 256, V tiles
  padded to an 80-col pitch); the two exact tiles use fp16 PV.  PVs
  trail the score pipeline by PIPE pairs so exp latency is hidden.
- Copy-out split ACT/DVE by the load balancer; host divides by the
  denominator row and transposes back.
- Warm-up matmuls stream a zero fodder region so every score-PSUM
  bank holds bounded values before the first stair fast-exp reads a
  gap column.
"""
import os
import sys

sys.path.insert(0, "/opt/trn_rl_repo")

import numpy as np

B, H, S, D = 4, 16, 2048, 64
NCORES = 8
HPC = (B * H) // NCORES        # heads per core = 8
QB = 1024                      # q-block width (acc/copy granularity)
CH = 512                       # score/exp/PV chunk width (1 PSUM bank)
NQB = S // QB                  # q blocks per head = 2
V8K = 80                       # fp8 V k-tile pitch (DoubleRow LDW
                               # needs tile stride % 16 == 0)
NKT = S // 128                 # 128-wide k-tiles per head = 16
V8COLS = NKT * V8K             # 16*80 = 1280 (fp8 V, all k-tiles)
V16COLS = 2 * (D + 1)          # 130 (fp16 V, k-tiles 0-1 only)
KTC = 1024                     # kt cols per head (even top / odd bottom)
QTC = 2048                     # qt cols per head (Q^T duplicated)
# CMB pair layout: ktA | ktB | qtA | qtB | v16A | v16B
OFF_KTA, OFF_KTB = 0, KTC
OFF_QTA, OFF_QTB = 2 * KTC, 2 * KTC + QTC
OFF_V16A = 2 * KTC + 2 * QTC
OFF_V16B = OFF_V16A + V16COLS
PAIR_COLS = OFF_V16B + V16COLS  # 6404
NPAIR = HPC // 2               # 4
SCALE = 1.0 / 8.0              # 1/sqrt(D)
PVB = 4                        # PV burst size; PVs trail scores by
                               # >= PVB pairs so exp latency is hidden

# All E values carry a 2^-4 factor (cancels in numerator/denominator):
# raw scores reach ~55, and exp(55/8)=963 would overflow fp8e4m3's 448
# max (the uint8 fast-exp would wrap into NaN/negative patterns).
EXP_BIAS = float(-4.0 * np.log(2.0))  # exp(x*SCALE + EXP_BIAS)
# Schraudolph fast-exp constants (fp8e4m3 pattern via saturating uint8,
# ~7.3% max rel err; stair-masked scores saturate to 0 = fp8 +0.0).
FE8_A = float(8.0 * np.log2(np.e) * SCALE)
FE8_B = float(56.0 - 0.37 - 32.0)
MASK_B = -1000.0               # stair bias above the diagonal
A16 = True                     # exact ACT exp for k-tiles 0,1 of qb0

# engine cost model (ns): t = cols * NS_COL + NS_FIX
ACT_NS_COL, ACT_NS_FIX = 0.84, 290.0
DVE_NS_COL, DVE_NS_FIX = 1.05, 160.0

last_exec_time_ns = None

_prog_cache = {}


def _install_trace_hook():
    """Inject antenv.axon_hooks (missing from this image) so trace=True can
    capture NTFF profiles. Degrades silently if anything is unavailable."""
    import types

    try:
        import antenv

        if "antenv.axon_hooks" in sys.modules:
            return
        mod = types.ModuleType("antenv.axon_hooks")
        state = {"hook": None}
        mod.set_axon_ntff_profile_hook = lambda h: state.__setitem__("hook", h)
        mod.get_axon_ntff_profile_hook = lambda: state["hook"]
        sys.modules["antenv.axon_hooks"] = mod
        antenv.axon_hooks = mod
        from trn_agent_boot.trn_boot import _ntff_profile_via_ctypes

        hook = _ntff_profile_via_ctypes("/opt/axon/libaxon_pjrt.so")
        if hook is not None:
            mod.set_axon_ntff_profile_hook(hook)
    except Exception:
        pass


def _chunk_pairs(qb, c):
    """Tile pairs for chunk c of q-block qb: (ke, sge, sgo, diag)."""
    qc0 = qb * QB + c * CH
    kmax = qc0 // 128 + 4
    out = []
    for ke in range(0, kmax, 2):
        sge = max(0, 128 * ke - qc0)
        sgo = max(0, 128 * (ke + 1) - qc0)
        out.append((ke, sge, sgo, 128 * ke >= qc0))
    return out


def _plan_exp_engines():
    """Static engine plan.

    plan[(head, qb, c, ke)] -> 'a16' | 'act' | 'dve' per tile PAIR;
    cplan[(head, qb)] -> cols assigned to ACT for the copy-out (rest
    DVE).  Diag pairs are pinned to DVE (the stair bias lives in the
    fused scalar_tensor_tensor); off-diag pairs and the copy split
    balance modeled finish times.
    """
    load = {"act": 0.0, "dve": 0.0}

    def cost(eng, cols):
        return (cols * ACT_NS_COL + ACT_NS_FIX if eng == "act"
                else cols * DVE_NS_COL + DVE_NS_FIX)

    plan = {}
    cplan = {}
    for head in range(HPC):
        for qb in range(NQB):
            for c in range(QB // CH):
                for ke, sge, sgo, diag in _chunk_pairs(qb, c):
                    if A16 and qb == 0 and c == 0 and ke == 0:
                        plan[(head, qb, c, ke)] = "a16"
                        load["act"] += (cost("act", CH)
                                        + cost("act", CH - 128))
                    elif diag:
                        plan[(head, qb, c, ke)] = "dve"
                        load["dve"] += cost("dve", 2 * (CH - sge))
                    elif load["act"] + cost("act", 2 * CH) <= \
                            load["dve"] + cost("dve", 2 * CH):
                        plan[(head, qb, c, ke)] = "act"
                        load["act"] += cost("act", 2 * CH)
                    else:
                        plan[(head, qb, c, ke)] = "dve"
                        load["dve"] += cost("dve", 2 * CH)
            # copy-out split: give ACT enough cols to equalize
            gap = load["dve"] - load["act"]
            aa = int((QB * DVE_NS_COL + gap + DVE_NS_FIX - ACT_NS_FIX)
                     / (ACT_NS_COL + DVE_NS_COL))
            aa = max(0, min(QB, (aa // 64) * 64))
            cplan[(head, qb)] = aa
            if aa > 0:
                load["act"] += cost("act", aa)
            if aa < QB:
                load["dve"] += cost("dve", QB - aa)
    return plan, cplan


def _build_program():
    import concourse.bass as bass  # noqa: F401
    import concourse.mybir as mybir
    import concourse.tile as tile
    from concourse import bacc

    F16 = mybir.dt.float16
    F32 = mybir.dt.float32
    F8 = mybir.dt.float8e4
    U8 = mybir.dt.uint8
    EXP = mybir.ActivationFunctionType.Exp
    MULT = mybir.AluOpType.mult
    ADD = mybir.AluOpType.add
    DR = mybir.MatmulPerfMode.DoubleRow

    plan, cplan = _plan_exp_engines()

    nc = bacc.Bacc()
    CMB = nc.declare_dram_parameter(
        "CMB", [128, NPAIR * PAIR_COLS], F16, isOutput=False
    )
    VA8 = nc.declare_dram_parameter(
        "VA8", [128, HPC * V8COLS], F8, isOutput=False
    )
    TRI = nc.declare_dram_parameter("TRI", [128, 1408], F16, isOutput=False)
    OUT = nc.declare_dram_parameter("OUT", [HPC, D + 1, S], F16, isOutput=True)

    with tile.TileContext(nc) as tc:
        with (
            tc.tile_pool(name="cmbp", bufs=2) as cmbp,
            tc.tile_pool(name="singles", bufs=1) as singles,
            tc.tile_pool(name="etp16", bufs=2) as etp16,
            tc.tile_pool(name="etp8", bufs=10) as etp8,
            tc.tile_pool(name="obp", bufs=2) as obp,
            tc.tile_pool(name="stp", bufs=3, space="PSUM") as stp,
            tc.tile_pool(name="accp", bufs=1, space="PSUM") as accp,
        ):
            biast = singles.tile([128, 1], F32, tag="ebias")
            nc.gpsimd.memset(biast, EXP_BIAS)
            # memset-built zero fodder: the PE warm-up needs no DMA
            zw = singles.tile([128, 640], F16, tag="zw")
            nc.gpsimd.memset(zw, 0.0)
            trib = singles.tile([128, 1408], F16, tag="tri")
            # 0/1 causal stair for the two exact ACT tiles (applied
            # as a GpSimd multiply on the fp16 E tile)
            stairm = trib[:, 0:128]
            # per-slot stair bias for the fused DVE fast-exp:
            # slot 0: stair then FE8_B; slot 1: -1000 preamble (covers
            # the even tile's extra cols), stair, then FE8_B.
            stair2 = trib[:, 128:1408].rearrange("p (s c) -> p s c", s=2)

            va8b = singles.tile([128, HPC * V8COLS], F8, tag="va8")

            # PE warm-up: the HAM clock gate only un-throttles (1.2 ->
            # 2.4 GHz) under sustained matmul activity; streamed zeros
            # also leave every score-PSUM bank bounded before the first
            # stair fast-exp reads a stale gap column.
            for wi in range(9):
                wt = stp.tile([128, 2 * CH], F32, tag="st",
                              name=f"warm{wi}")
                nc.tensor.matmul(wt[:, 0:CH], zw[:, 0:128],
                                 zw[:, 128:640], start=True, stop=True)
                nc.tensor.matmul(wt[:, CH:2 * CH], zw[:, 0:128],
                                 zw[:, 128:640], start=True, stop=True)

            # stage pair 0 so head 0's first chunk can start early:
            # both heads' kt, head A's qb0 Q, the fp16 V tiles and the
            # first heads' fp8 V, then the remainder.
            cmbs = [cmbp.tile([128, PAIR_COLS], F16, tag="cmb",
                              name=f"cmb{p}") for p in range(NPAIR)]
            nc.sync.dma_start(out=cmbs[0][:, 0:2 * KTC],
                              in_=CMB[:, 0:2 * KTC])
            nc.sync.dma_start(out=trib, in_=TRI[:])
            nc.gpsimd.dma_start(
                out=cmbs[0][:, OFF_QTA:OFF_QTA + QB],
                in_=CMB[:, OFF_QTA:OFF_QTA + QB])
            nc.gpsimd.dma_start(
                out=cmbs[0][:, OFF_V16A:PAIR_COLS],
                in_=CMB[:, OFF_V16A:PAIR_COLS])
            nc.scalar.dma_start(
                out=va8b[:, 0:2 * V8COLS], in_=VA8[:, 0:2 * V8COLS]
            )
            for c0, c1 in (
                (OFF_QTA + QB, OFF_QTB),       # qtA qb1 cols
                (OFF_QTB, OFF_QTB + QB),       # qtB qb0
                (OFF_QTB + QB, OFF_V16A),      # qtB qb1
            ):
                nc.sync.dma_start(out=cmbs[0][:, c0:c1],
                                  in_=CMB[:, c0:c1])
            nc.sync.dma_start(
                out=va8b[:, 2 * V8COLS:], in_=VA8[:, 2 * V8COLS:]
            )

            # copy-out closures deferred into the NEXT block's pair
            # stream so they never head-of-line-block the engines'
            # strict FIFO queues while waiting for the PV flush
            carry = []

            for pair in range(NPAIR):
                cmb = cmbs[pair]
                for sub in range(2):
                    if sub == 1 and pair + 1 < NPAIR:
                        # prefetch the next pair on the idle GpSimd
                        # DMA queue: issued at the second head so it
                        # never contends with this pair's own staging,
                        # overlaps ~16us of compute, and is not stuck
                        # behind OUT drains on the Sync queue
                        nc.gpsimd.dma_start(
                            out=cmbs[pair + 1],
                            in_=CMB[:, (pair + 1) * PAIR_COLS:
                                    (pair + 2) * PAIR_COLS],
                        )
                    head = 2 * pair + sub
                    kt = cmb[:, OFF_KTA + sub * KTC:OFF_KTA + (sub + 1) * KTC]
                    qt = cmb[:, OFF_QTA + sub * QTC:OFF_QTA + (sub + 1) * QTC]
                    v16off = OFF_V16A + sub * V16COLS
                    va16 = cmb[:, v16off:v16off + V16COLS].rearrange(
                        "p (t c) -> p t c", t=2
                    )
                    va8 = va8b[:, head * V8COLS:(head + 1) * V8COLS
                               ].rearrange("p (t c) -> p t c", t=NKT)

                    for qb in range(NQB):
                        q0 = QB * qb
                        acc = accp.tile(
                            [D + 1, QB], F32, tag="acc",
                            name=f"acc_h{head}_qb{qb}",
                        )
                        e16s = {}
                        pend = []      # pv work awaiting issue

                        def do_st_pair(c, ke, sge, sgo, diag,
                                       first, last):
                            """Row-tiled concurrent score pair + exp."""
                            qc0 = q0 + c * CH
                            tcol = 64 * ke
                            kt_e = kt[0:64, tcol:tcol + 128]
                            kt_o = kt[64:128, tcol:tcol + 128]
                            nm = f"h{head}_q{qb}{c}_k{ke}"
                            st = stp.tile([128, 2 * CH], F32, tag="st",
                                          name=f"st_{nm}")
                            nc.tensor.matmul(
                                st[:, sge:CH], kt_e,
                                qt[0:64, qc0 + sge:qc0 + CH],
                                start=True, stop=True,
                            )
                            nc.tensor.matmul(
                                st[:, CH + sgo:2 * CH], kt_o,
                                qt[64:128, qc0 + sgo:qc0 + CH],
                                start=True, stop=True,
                            )
                            st3 = st.rearrange("p (s c) -> p s c", s=2)
                            if A16 and qb == 0 and c == 0 and ke == 0:
                                # exact tiles: unmasked ACT exp, then
                                # the causal stair applied as a 0/1
                                # multiply on the idle GpSimd engine
                                # (keeps the rank-128 mask matmul off
                                # the PE)
                                for ki in (0, 1):
                                    sg = 128 * ki
                                    et = etp16.tile(
                                        [128, CH], F16, tag="et16",
                                        name=f"et_h{head}_k{ki}",
                                    )
                                    nc.scalar.activation(
                                        et[:, sg:CH],
                                        st[:, ki * CH + sg:(ki + 1) * CH],
                                        EXP, bias=biast, scale=SCALE,
                                    )
                                    nc.gpsimd.tensor_tensor(
                                        et[:, sg:sg + 128],
                                        et[:, sg:sg + 128],
                                        stairm, MULT,
                                    )
                                    e16s[ki] = (et, sg)
                                return
                            e8 = etp8.tile([128, 2, CH], F8, tag="et8",
                                           name=f"e8_{nm}")
                            eng = plan[(head, qb, c, ke)]
                            if diag:
                                # fused stair fast-exp over both slots:
                                # (st*A)+stair2, saturating uint8 ->
                                # fp8 pattern; slot 1's -1000 preamble
                                # zeroes the gap columns.
                                nc.vector.scalar_tensor_tensor(
                                    e8[:, :, sge:CH].bitcast(U8),
                                    st3[:, :, sge:CH],
                                    FE8_A, stair2[:, :, 0:CH - sge],
                                    MULT, ADD,
                                )
                            elif eng == "dve":
                                nc.vector.tensor_scalar(
                                    e8.bitcast(U8), st3,
                                    FE8_A, FE8_B, MULT, ADD,
                                )
                            else:
                                nc.scalar.activation(
                                    e8, st3, EXP,
                                    bias=biast, scale=SCALE,
                                )
                            pend.append((c, ke, sge, e8, first, last))

                        def do_pv16():
                            for ki in (0, 1):
                                et, sg = e16s.pop(ki)
                                nc.tensor.matmul(
                                    acc[:, sg:CH], va16[:, ki, :],
                                    et[:, sg:CH],
                                    start=(ki == 0), stop=False,
                                )

                        def do_pv8(item):
                            c, ke, sge, e8, first, last = item
                            c0 = c * CH
                            va_k = va8[:, ke:ke + 2, 0:D + 1]
                            nc.tensor.matmul(
                                acc[:, c0 + sge:c0 + CH], va_k,
                                e8[:, :, sge:CH],
                                start=first, stop=last,
                                perf_mode=DR,
                            )

                        pcount = 0
                        for c in range(QB // CH):
                            cps = _chunk_pairs(qb, c)
                            for t, (ke, sge, sgo, diag) in enumerate(cps):
                                # first PV of the chunk clears PSUM;
                                # in qb0 chunk0 that's the PV16 pair
                                first = t == 0 and not (
                                    A16 and qb == 0 and c == 0)
                                last = t == len(cps) - 1
                                do_st_pair(c, ke, sge, sgo, diag,
                                           first, last)
                                pcount += 1
                                # the deferred copy must be emitted
                                # before this block's first PV writes
                                # the (single-buffered) acc: before
                                # PV16 (pair 2) in qb0, before the
                                # first PV burst (pair 6) in qb1
                                if carry and pcount == 2:
                                    carry.pop(0)()
                                if A16 and qb == 0 and c == 0 and ke == 2:
                                    do_pv16()
                                if len(pend) >= 2 * PVB:
                                    for _ in range(PVB):
                                        do_pv8(pend.pop(0))
                        while pend:
                            do_pv8(pend.pop(0))

                        def emit_copy(acc=acc, head=head, qb=qb, q0=q0):
                            ob = obp.tile(
                                [D + 1, QB], F16, tag="ob",
                                name=f"ob_h{head}_qb{qb}",
                            )
                            aa = cplan[(head, qb)]
                            if aa > 0:
                                nc.scalar.copy(ob[:, 0:aa], acc[:, 0:aa])
                            if aa < QB:
                                nc.vector.tensor_copy(
                                    ob[:, aa:QB], acc[:, aa:QB]
                                )
                            nc.sync.dma_start(
                                out=OUT[head, :, q0:q0 + QB], in_=ob,
                            )
                        carry.append(emit_copy)
            while carry:
                carry.pop(0)()
    nc.finalize()
    return nc


def _get_program():
    if "nc" not in _prog_cache:
        _prog_cache["nc"] = _build_program()
    return _prog_cache["nc"]


def kernel(q, k, v, mask):
    global last_exec_time_ns
    q = np.asarray(q, dtype=np.float32)
    k = np.asarray(k, dtype=np.float32)
    v = np.asarray(v, dtype=np.float32)
    mask = np.asarray(mask).astype(bool)

    # This kernel specializes the causal (lower-triangular) mask from the
    # module; for any other mask fall back to a host reference.
    tril = np.tril(np.ones((S, S), dtype=bool))
    if mask.shape != (1, 1, S, S) or not np.array_equal(mask[0, 0], tril):
        scores = np.einsum("bhqd,bhkd->bhqk", q, k) / np.sqrt(np.float32(D))
        scores = np.where(mask, scores, -np.inf)
        m = scores.max(-1, keepdims=True)
        e = np.exp(scores - m)
        return (np.einsum("bhqk,bhkd->bhqd", e / e.sum(-1, keepdims=True), v)
                .astype(np.float32))

    _install_trace_hook()
    import ml_dtypes
    from concourse.bass_utils import run_bass_kernel_spmd

    nc = _get_program()

    F8NP = ml_dtypes.float8_e4m3fn
    qf = q.reshape(B * H, S, D).astype(np.float16)
    kf = k.reshape(B * H, S, D).astype(np.float16)
    vf = v.reshape(B * H, S, D).astype(np.float16)

    tri_np = np.zeros((128, 1408), dtype=np.float16)
    # 0/1 causal stair: keep k_rel (p) <= q_rel (j)
    pp = np.arange(128)[:, None]
    jj = np.arange(128)[None, :]
    tri_np[:, 0:128] = (pp <= jj).astype(np.float16)
    # stair bias slots: [p, j] = FE8_B if p <= j else MASK_B
    p = np.arange(128)[:, None]
    j = np.arange(128)[None, :]
    stair = np.where(p <= j, np.float16(FE8_B), np.float16(MASK_B))
    s0 = np.full((128, 640), np.float16(FE8_B), dtype=np.float16)
    s0[:, 0:128] = stair
    s1 = np.full((128, 640), np.float16(FE8_B), dtype=np.float16)
    s1[:, 0:128] = np.float16(MASK_B)
    s1[:, 128:256] = stair
    tri_np[:, 128:768] = s0
    tri_np[:, 768:1408] = s1

    def _kt_pack(h):
        kth = np.zeros((128, KTC), dtype=np.float16)
        kT = kf[h].T  # [64, 2048]
        for t in range(NKT // 2):
            kth[0:64, 128 * t:128 * (t + 1)] = kT[:, 256 * t:256 * t + 128]
            kth[64:128, 128 * t:128 * (t + 1)] = \
                kT[:, 256 * t + 128:256 * t + 256]
        return kth

    in_maps = []
    for core in range(NCORES):
        pairs = []
        va8s = []
        for p_ in range(NPAIR):
            hA = core * HPC + 2 * p_
            hB = hA + 1
            ktA, ktB = _kt_pack(hA), _kt_pack(hB)
            qtA = np.concatenate([qf[hA].T, qf[hA].T], axis=0)  # dup
            qtB = np.concatenate([qf[hB].T, qf[hB].T], axis=0)
            v16s = []
            for h in (hA, hB):
                vt = vf[h].reshape(NKT, 128, D).transpose(1, 0, 2)
                va = np.concatenate(
                    [vt, np.ones((128, NKT, 1), dtype=np.float16)], axis=2
                )  # [128, NKT, 65]
                v16s.append(va[:, 0:2, :].reshape(128, V16COLS))
                va8p = np.zeros((128, NKT, V8K), dtype=F8NP)
                va8p[:, :, 0:D + 1] = va.astype(F8NP)
                va8s.append(va8p.reshape(128, V8COLS))
            pairs.append(
                np.concatenate([ktA, ktB, qtA, qtB, v16s[0], v16s[1]],
                               axis=1)
            )
        cmb = np.ascontiguousarray(np.concatenate(pairs, axis=1))
        va8 = np.ascontiguousarray(np.concatenate(va8s, axis=1))
        in_maps.append({"CMB": cmb, "VA8": va8, "TRI": tri_np})

    trace = bool(os.environ.get("ATTN_TRACE"))
    res = run_bass_kernel_spmd(
        nc, in_maps, list(range(NCORES)), trace=trace
    )
    last_exec_time_ns = res.exec_time_ns

    out = np.empty((B * H, S, D), dtype=np.float32)
    for core in range(NCORES):
        acc = res.results[core]["OUT"].astype(np.float32)  # [HPC, 65, S]
        o = acc[:, :D, :] / acc[:, D:D + 1, :]
        out[core * HPC:(core + 1) * HPC] = o.transpose(0, 2, 1)
    return out.reshape(B, H, S, D)
